# revision 4
# baseline (speedup 1.0000x reference)
"""ContrastiveCenterLoss Trainium2 Bass kernel.

Math
----
reference:  dis[b,c] = cos(hidden_b, center_c);  intra_b = dis[b, y_b];
            inter_b  = (sum_c dis[b,c] - intra_b) / (C-1)
            loss     = mean(1 - intra_b + inter_b)

Folded form used here (exact algebraic identities):
    cn_c    = fc_c / max(||fc_c||, eps)         (normalized centers)
    s       = sum_c cn_c
    invh_b  = 1 / max(||hidden_b||, eps)
    u_b     = hidden_b . cn_{y_b}               (gathered bf16 cn row)
    w       = sum_b invh_b * hidden_b           (PE matmul accumulation)
    loss    = 1 + [w.s - C * sum_b invh_b*u_b] / (B * (C-1))

The rowsum side collapses into one 128-d vector w (distributivity), so the
only per-sample work is two dot products (sumsq and u) plus a 256B-row
gather cn[y_b] — no [B,C] cosine matrix. The kernel is memory-bound: read
`hidden` once plus a bf16 gather.

Sharding: data-parallel over batch across 8 cores (4096 rows/core),
feature_center replicated; host sums the per-core partials:
    loss = 1 + (sum ws_c - C * sum partial_c) / (B*(C-1))

Engine budget per core: DVE runs the u dot products (fused mult+reduce TTR)
and half the q reductions; ACT does bulk squares; gpsimd does the gather
descriptor generation plus the other half of q via fused square+accum; PE
accumulates w. trn2 PE instructions allow only ONE sync wait, so PE operand
producers are kept on a single engine (DVE), with tiny self-referencing
"observer" matmuls to absorb the DMA-completion ticks first.
"""

import sys

sys.path.insert(0, "/opt/trn_rl_repo")

from contextlib import ExitStack

import numpy as np

import concourse.bass as bass
import concourse.tile as tile
from concourse import bacc, mybir
from concourse.bass import IndirectOffsetOnAxis
from concourse import library_config

B, C, D = 32768, 1000, 128
NCORES = 8
BS = B // NCORES          # 4096 rows per core
NT = BS // 128            # 32 batch tiles of 128 rows
CT = 8                    # center tiles
CP = C // CT              # 125 centers per tile
EPS = 1e-8
F32 = mybir.dt.float32
BF16 = mybir.dt.bfloat16
HCHUNKS = 8               # hidden-load / compute pipeline chunks
HTPC = NT // HCHUNKS
GCHUNKS = 2               # gather chunks
GTPC = NT // GCHUNKS
GPSIMD_Q_CHUNKS = 2       # earliest chunks' q on gpsimd, rest ACT+DVE
GPSIMD_U_TILES = 10       # trailing u tiles offloaded to gpsimd

_CACHED_NC = None


def build_nc() -> bass.Bass:
    AF = mybir.ActivationFunctionType
    OP = mybir.AluOpType

    nc = bacc.Bacc(dynamic_dma_scratch_size=65536)
    hidden = nc.dram_tensor("hidden", [BS, D], F32, kind="ExternalInput")
    fc = nc.dram_tensor("fc", [C, D], F32, kind="ExternalInput")
    yidx = nc.dram_tensor("yidx", [128, NT], mybir.dt.int32, kind="ExternalInput")
    out_res = nc.dram_tensor("res", [128, 2], F32, kind="ExternalOutput")
    cn_dram = nc.dram_tensor("cn_table", [C, D], BF16)  # internal scratch

    with tile.TileContext(nc) as tc, ExitStack() as ctx:
        singles = ctx.enter_context(tc.tile_pool(name="singles", bufs=1))
        work = ctx.enter_context(tc.tile_pool(name="work", bufs=4))
        psum = ctx.enter_context(tc.tile_pool(name="psum", bufs=1, space="PSUM"))

        # ---------------- phase 0: normalized-center table ----------------
        # fc rows (t*CP + p) -> fc_sb[p, t, :]
        fc_sb = singles.tile([CP, CT, D], F32)
        fc_src = fc[:, :].rearrange("(t p) d -> p t d", t=CT)
        nc.sync.dma_start(out=fc_sb[:, 0 : CT // 2, :], in_=fc_src[:, 0 : CT // 2, :])
        nc.scalar.dma_start(
            out=fc_sb[:, CT // 2 :, :], in_=fc_src[:, CT // 2 :, :]
        )
        # Preload the ACT sqrt/square function table early so the first real
        # activation doesn't pay the 1.3us table load on the critical chain.
        warm = singles.tile([128, 1], F32)
        nc.scalar.activation(out=warm, in_=warm, func=AF.Sqrt, scale=0.0, bias=1.0)

        # q_c[p, t] = ||fc row||^2, fused square+reduce per center tile (DVE)
        q_c = singles.tile([CP, CT], F32)
        for t in range(CT):
            prodc = work.tile([CP, D], F32, tag="prodc")
            nc.vector.scalar_tensor_tensor(
                out=prodc,
                in0=fc_sb[:, t, :],
                scalar=1.0,
                op0=OP.mult,
                in1=fc_sb[:, t, :],
                op1=OP.mult,
                accum_out=q_c[:, t : t + 1],
            )
        rt_c = singles.tile([CP, CT], F32)
        nc.scalar.activation(out=rt_c, in_=q_c, func=AF.Sqrt)
        nc.vector.tensor_scalar_max(out=rt_c, in0=rt_c, scalar1=EPS)
        inv_c = singles.tile([CP, CT], F32)
        nc.vector.reciprocal(out=inv_c, in_=rt_c)

        # cn = fc * inv_c, written directly as bf16 (table for gather + matmul)
        cn_bf = singles.tile([CP, CT, D], BF16)
        cn_dst = cn_dram[:, :].rearrange("(t p) d -> p t d", t=CT)
        for hh in range(2):
            t0, t1 = hh * (CT // 2), (hh + 1) * (CT // 2)
            nc.vector.tensor_tensor(
                out=cn_bf[:, t0:t1, :],
                in0=fc_sb[:, t0:t1, :],
                in1=inv_c[:, t0:t1].broadcast_to([CP, CT // 2, D]),
                op=OP.mult,
            )
            # store the table halves for the gather (ACT HWDGE queue)
            nc.scalar.dma_start(out=cn_dst[:, t0:t1, :], in_=cn_bf[:, t0:t1, :])

        # s[d] = sum_c cn[c, d]: ones-matmul partition reduction (off-path).
        ones_col = singles.tile([128, 1], BF16)
        nc.vector.memset(ones_col, 1.0)
        s_wide_ps = psum.tile([1, CT, D], F32)
        half = CT // 2  # keep each matmul's free size at 512 (one PSUM bank)
        for h in range(2):
            nc.tensor.matmul(
                out=s_wide_ps[:, h * half : (h + 1) * half, :],
                lhsT=ones_col[:CP, :],
                rhs=cn_bf[:, h * half : (h + 1) * half, :],
                start=True,
                stop=True,
            )
        s_sb = singles.tile([1, D], F32)
        nc.vector.tensor_reduce(
            out=s_sb[:, :],
            in_=s_wide_ps[:, :, :].rearrange("o t d -> o d t"),
            axis=mybir.AxisListType.X,
            op=OP.add,
        )

        # ---------------- main ----------------
        # h_all[p, i, :] = hidden[32*p + i, :]
        h_all = singles.tile([128, NT, D], F32)
        h_src = hidden[:, :].rearrange("(p i) d -> p i d", p=128)
        yi = singles.tile([128, NT], mybir.dt.int32)
        nc.sync.dma_start(out=yi[:, :], in_=yidx[:, :])
        for k in range(HCHUNKS):
            j0, j1 = k * HTPC, (k + 1) * HTPC
            nc.sync.dma_start(out=h_all[:, j0:j1, :], in_=h_src[:, j0:j1, :])

        # gather cn[y] (bf16): HW-indirect DMA, one call per 128-row tile.
        # Descriptor expansion happens in the DMA engine (no gpsimd work).
        cng = singles.tile([128, NT, D], BF16)
        for j in range(NT):
            nc.gpsimd.indirect_dma_start(
                out=cng[:, j, :],
                out_offset=None,
                in_=cn_dram[:, :],
                in_offset=IndirectOffsetOnAxis(ap=yi[:, j : j + 1], axis=0),
            )

        # PE "observer" matmuls: absorb each h-chunk's DMA tick so later
        # w-matmuls carry only a single (DVE) sync wait.
        junk_ps = psum.tile([1, 1], F32)
        for k in range(HCHUNKS):
            col = h_all[:, k * HTPC, 0:1]
            nc.tensor.matmul(
                out=junk_ps[:, :], lhsT=col, rhs=col, start=True, stop=True
            )

        q_all = singles.tile([128, NT], F32)
        z_all = singles.tile([128, NT], F32)
        inv_h = singles.tile([128, NT], F32)
        sq_late = singles.tile([128, NT, D], F32)
        w_ps = psum.tile([1, D], F32)

        # ---- phase A: q = ||h||^2 per chunk (bulk ACT square + DVE reduce) ----
        for k in range(HCHUNKS):
            j0, j1 = k * HTPC, (k + 1) * HTPC
            nc.scalar.activation(
                out=sq_late[:, j0:j1, :], in_=h_all[:, j0:j1, :], func=AF.Square
            )
            nc.vector.tensor_reduce(
                out=q_all[:, j0:j1],
                in_=sq_late[:, j0:j1, :],
                axis=mybir.AxisListType.X,
                op=OP.add,
            )

        # ---- phase B: invh = 1/max(sqrt(q), eps) per chunk ----
        for k in range(HCHUNKS):
            j0, j1 = k * HTPC, (k + 1) * HTPC
            nc.scalar.activation(
                out=inv_h[:, j0:j1], in_=q_all[:, j0:j1], func=AF.Sqrt
            )
            nc.vector.tensor_scalar_max(
                out=inv_h[:, j0:j1], in0=inv_h[:, j0:j1], scalar1=EPS
            )
            nc.vector.reciprocal(out=inv_h[:, j0:j1], in_=inv_h[:, j0:j1])

        # ---- phase C: u = h . cn[y] per tile (DVE fused mult+accum) ----
        for j in range(NT):
            prod = work.tile([128, D], F32, tag="prod")
            nc.vector.scalar_tensor_tensor(
                out=prod,
                in0=h_all[:, j, :],
                scalar=1.0,
                op0=OP.mult,
                in1=cng[:, j, :],
                op1=OP.mult,
                accum_out=z_all[:, j : j + 1],
            )

        # ---- phase D: w += invh_b * h_b (PE accumulation) ----
        for j in range(NT):
            nc.tensor.matmul(
                out=w_ps[:, :],
                lhsT=inv_h[:, j : j + 1],
                rhs=h_all[:, j, :],
                start=(j == 0),
                stop=(j == NT - 1),
                skip_group_check=True,
            )

        # ---------------- tail ----------------
        res_sb = singles.tile([128, 2], F32)
        nc.vector.memset(res_sb, 0.0)
        # ws = w . s  -> res[0, 1] (ready as soon as phase D finishes)
        wprod = singles.tile([1, D], F32)
        nc.vector.scalar_tensor_tensor(
            out=wprod,
            in0=w_ps[:, :],
            scalar=1.0,
            op0=OP.mult,
            in1=s_sb[:, :],
            op1=OP.mult,
            accum_out=res_sb[0:1, 1:2],
        )
        # partial[p] = sum_j z[p,j]*invh[p,j] -> res[:, 0] (one short op)
        vprod = singles.tile([128, NT], F32, tag="vprod")
        nc.vector.scalar_tensor_tensor(
            out=vprod,
            in0=z_all[:, :],
            scalar=1.0,
            op0=OP.mult,
            in1=inv_h[:, :],
            op1=OP.mult,
            accum_out=res_sb[:, 0:1],
        )
        nc.sync.dma_start(out=out_res[:, 0:2], in_=res_sb[:, 0:2])

    return nc


def _get_nc() -> bass.Bass:
    global _CACHED_NC
    if _CACHED_NC is None:
        _CACHED_NC = build_nc()
        _CACHED_NC.finalize()
    return _CACHED_NC


def _wrap_idx(y_shard: np.ndarray) -> np.ndarray:
    """Index layout matching h_all[p, j, :] = hidden[32p + j]:
    yidx[p, j] = y_shard[32p + j], int32."""
    return np.ascontiguousarray(y_shard.reshape(128, NT).astype(np.int32))


def make_in_maps(hidden, feature_center, y):
    hidden = np.ascontiguousarray(np.asarray(hidden), dtype=np.float32)
    fc = np.ascontiguousarray(np.asarray(feature_center), dtype=np.float32)
    y32 = np.asarray(y).astype(np.int32)
    in_maps = []
    for c in range(NCORES):
        hs = hidden[c * BS : (c + 1) * BS]
        ys = _wrap_idx(y32[c * BS : (c + 1) * BS])
        in_maps.append({"hidden": hs, "fc": fc, "yidx": ys})
    return in_maps


def finish(results) -> np.ndarray:
    """results: list of dicts with 'res' [128,2]: col0 partials, [0,1] ws."""
    tot_u = 0.0
    tot_ws = 0.0
    for r in results:
        res = np.asarray(r["res"], dtype=np.float64)
        tot_u += res[:, 0].sum()
        tot_ws += res[0, 1]
    return np.float32(1.0 + (tot_ws - C * tot_u) / (B * (C - 1)))


def kernel(hidden, feature_center, y) -> np.ndarray:
    from concourse.bass_utils import run_bass_kernel_spmd

    in_maps = make_in_maps(hidden, feature_center, y)
    nc = _get_nc()
    res = run_bass_kernel_spmd(nc, in_maps, core_ids=list(range(NCORES)))
    return finish(res.results)



# revision 7
# speedup vs baseline: 1.3846x; 1.3846x over previous
"""ContrastiveCenterLoss Trainium2 Bass kernel — gather-free formulation.

Math
----
reference:  dis[b,c] = cos(hidden_b, center_c);  intra_b = dis[b, y_b];
            inter_b  = (sum_c dis[b,c] - intra_b) / (C-1)
            loss     = mean(1 - intra_b + inter_b)

Folded form (exact algebraic identities):
    cn_c   = fc_c / ||fc_c||          (normalized centers)
    s      = sum_c cn_c
    X_b    = hidden_b / ||hidden_b||
    G_c    = sum_{b: y_b = c} X_b     (class-conditional sums)
    loss   = 1 + sum_c G_c . (s - C*cn_c) / (B * (C-1))

because sum_b X_b.s = w.s (the rowsum/inter side) and
sum_c G_c.cn_c = sum_b intra contributions. No [B,C] matrix, no
per-sample gather of cn rows.

G is computed on the PE as one-hot matmuls: the HOST relabels classes
into 8 windows of 125 (balancing sample counts per window) and permutes
each core's batch shard so tile j (128 samples) only holds classes of
window j//5. Then per tile:  G_window += onehot[b,125]^T @ X[b,128].
The one-hot is built on-chip from per-slot local ids (is_equal vs an
iota row). Pad slots duplicate a real row with lid=-1 (never matches),
so they contribute exactly nothing.

Sharding: data-parallel over batch across 8 cores, fc replicated; host
sums the per-core partial reductions.
"""

import sys

sys.path.insert(0, "/opt/trn_rl_repo")

from contextlib import ExitStack

import numpy as np

import concourse.bass as bass
import concourse.tile as tile
from concourse import bacc, mybir

B, C, D = 32768, 1000, 128
NCORES = 8
W = 8                     # class windows
CPW = C // W              # 125 classes per window
TPW = 5                   # tiles per window
NT = W * TPW              # 40 tiles of 128 slots per core
BS_PAD = NT * 128         # 5120 padded slots per core
NG = 4                    # compute pipeline groups
TPG = NT // NG            # 10 tiles per group
F32 = mybir.dt.float32
BF16 = mybir.dt.bfloat16

_CACHED_NC = None


def build_nc() -> bass.Bass:
    AF = mybir.ActivationFunctionType
    OP = mybir.AluOpType

    nc = bacc.Bacc()
    hidden = nc.dram_tensor("hidden", [BS_PAD, D], F32, kind="ExternalInput")
    fc = nc.dram_tensor("fc", [C, D], F32, kind="ExternalInput")
    lid_t = nc.dram_tensor("lid", [128, NT], F32, kind="ExternalInput")
    iota_t = nc.dram_tensor("iota", [128, CPW], F32, kind="ExternalInput")
    out_res = nc.dram_tensor("res", [128, 1], F32, kind="ExternalOutput")

    with tile.TileContext(nc) as tc, ExitStack() as ctx:
        singles = ctx.enter_context(tc.tile_pool(name="singles", bufs=1))
        psum = ctx.enter_context(tc.tile_pool(name="psum", bufs=1, space="PSUM"))

        # ---------------- DMA in ----------------
        # tiny tensors + fc on the scalar queue; hidden chunks on sync.
        lid_sb = singles.tile([128, NT], F32)
        iota_sb = singles.tile([128, CPW], F32)
        nc.scalar.dma_start(out=lid_sb, in_=lid_t[:, :])
        nc.scalar.dma_start(out=iota_sb, in_=iota_t[:, :])
        fc_sb = singles.tile([CPW, W, D], F32)
        fc_src = fc[:, :].rearrange("(t p) d -> p t d", t=W)
        nc.scalar.dma_start(out=fc_sb[:, 0 : W // 2, :], in_=fc_src[:, 0 : W // 2, :])
        nc.scalar.dma_start(out=fc_sb[:, W // 2 :, :], in_=fc_src[:, W // 2 :, :])

        h_all = singles.tile([128, NT, D], F32)
        h_src = hidden[:, :].rearrange("(p i) d -> p i d", p=128)
        HCH = 8
        HTPC = NT // HCH
        for k in range(HCH):
            j0, j1 = k * HTPC, (k + 1) * HTPC
            nc.sync.dma_start(out=h_all[:, j0:j1, :], in_=h_src[:, j0:j1, :])

        # Preload the ACT sqrt/square table early (1.3us table load).
        warm = singles.tile([128, 1], F32)
        nc.scalar.activation(out=warm, in_=warm, func=AF.Sqrt, scale=0.0, bias=1.0)

        # ---------------- one-hots (independent of hidden) ----------------
        # oh[p, j, c] = 1.0 if lid[p, j] == c else 0.0   (bf16)
        oh = singles.tile([128, NT, CPW], BF16)
        for g in range(NG):
            j0, j1 = g * TPG, (g + 1) * TPG
            eng = nc.vector
            eng.tensor_tensor(
                out=oh[:, j0:j1, :],
                in0=lid_sb[:, j0:j1].broadcast_to([128, TPG, CPW]),
                in1=iota_sb.unsqueeze(1).broadcast_to([128, TPG, CPW]),
                op=OP.is_equal,
            )

        # ---------------- normalized centers ----------------
        sq_c = singles.tile([CPW, W, D], BF16)
        nc.scalar.activation(out=sq_c, in_=fc_sb, func=AF.Square)
        q_c = singles.tile([CPW, W], F32)
        nc.vector.tensor_reduce(
            out=q_c, in_=sq_c, axis=mybir.AxisListType.X, op=OP.add
        )
        rt_c = singles.tile([CPW, W], F32)
        nc.scalar.activation(out=rt_c, in_=q_c, func=AF.Sqrt)
        inv_c = singles.tile([CPW, W], F32)
        nc.vector.reciprocal(out=inv_c, in_=rt_c)

        cn_bf = singles.tile([CPW, W, D], BF16)
        nc.vector.tensor_tensor(
            out=cn_bf,
            in0=fc_sb,
            in1=inv_c.broadcast_to([CPW, W, D]),
            op=OP.mult,
        )

        # s_rep[p, d] = sum_c cn[c, d] replicated on all 128 partitions:
        # 8 accumulating ones-matmuls (off critical path, PE idle early).
        ones_bf = singles.tile([CPW, 128], BF16)
        nc.vector.memset(ones_bf, 1.0)
        s_ps = psum.tile([128, D], F32)
        for t in range(W):
            nc.tensor.matmul(
                out=s_ps,
                lhsT=ones_bf,
                rhs=cn_bf[:, t, :],
                start=(t == 0),
                stop=(t == W - 1),
                skip_group_check=True,
            )

        # T = s - C*cn  (f32), the per-class loss weights
        T_sb = singles.tile([CPW, W, D], F32)
        nc.vector.scalar_tensor_tensor(
            out=T_sb,
            in0=cn_bf,
            scalar=float(-C),
            op0=OP.mult,
            in1=s_ps[0:CPW, :].unsqueeze(1).broadcast_to([CPW, W, D]),
            op1=OP.add,
        )

        # ---------------- main pipeline over 4 groups ----------------
        sq = singles.tile([128, NT, D], BF16)
        q_all = singles.tile([128, NT], F32)
        rt_all = singles.tile([128, NT], F32)
        invh = singles.tile([128, NT], F32)
        x_bf = singles.tile([128, NT, D], BF16)
        g_ps = psum.tile([CPW, W, D], F32)

        for g in range(NG):
            j0, j1 = g * TPG, (g + 1) * TPG
            nc.scalar.activation(
                out=sq[:, j0:j1, :], in_=h_all[:, j0:j1, :], func=AF.Square
            )
            nc.vector.tensor_reduce(
                out=q_all[:, j0:j1],
                in_=sq[:, j0:j1, :],
                axis=mybir.AxisListType.X,
                op=OP.add,
            )
            nc.scalar.activation(
                out=rt_all[:, j0:j1], in_=q_all[:, j0:j1], func=AF.Sqrt
            )
            nc.vector.reciprocal(out=invh[:, j0:j1], in_=rt_all[:, j0:j1])
            # X = h / ||h||  in bf16 (PE operand)
            eng = nc.vector
            eng.tensor_tensor(
                out=x_bf[:, j0:j1, :],
                in0=h_all[:, j0:j1, :],
                in1=invh[:, j0:j1].broadcast_to([128, TPG, D]),
                op=OP.mult,
            )
            # G_window += onehot^T @ X   per tile
            for j in range(j0, j1):
                w = j // TPW
                nc.tensor.matmul(
                    out=g_ps[:, w, :],
                    lhsT=oh[:, j, :],
                    rhs=x_bf[:, j, :],
                    start=(j % TPW == 0),
                    stop=(j % TPW == TPW - 1),
                    skip_group_check=True,
                )

        # ---------------- tail ----------------
        res_sb = singles.tile([128, 1], F32)
        nc.vector.memset(res_sb, 0.0)
        scratch = singles.tile([CPW, W, D], F32)
        nc.vector.scalar_tensor_tensor(
            out=scratch,
            in0=g_ps,
            scalar=1.0,
            op0=OP.mult,
            in1=T_sb,
            op1=OP.mult,
            accum_out=res_sb[0:CPW, 0:1],
        )
        nc.sync.dma_start(out=out_res[:, :], in_=res_sb)

    return nc


def _get_nc() -> bass.Bass:
    global _CACHED_NC
    if _CACHED_NC is None:
        _CACHED_NC = build_nc()
        _CACHED_NC.finalize()
    return _CACHED_NC


def make_in_maps(hidden, feature_center, y):
    hidden = np.ascontiguousarray(np.asarray(hidden), dtype=np.float32)
    fc = np.ascontiguousarray(np.asarray(feature_center), dtype=np.float32)
    y64 = np.asarray(y).astype(np.int64)

    counts = np.bincount(y64, minlength=C)
    order = np.argsort(-counts, kind="stable")
    # Greedy: heaviest class first into the lightest window with room.
    wsum = np.zeros(W, dtype=np.int64)
    wlen = np.zeros(W, dtype=np.int64)
    relabel = np.empty(C, dtype=np.int64)
    for c in order:
        cands = np.nonzero(wlen < CPW)[0]
        w = cands[np.argmin(wsum[cands])]
        relabel[c] = w * CPW + wlen[w]
        wlen[w] += 1
        wsum[w] += counts[c]

    fc_prime = np.empty_like(fc)
    fc_prime[relabel] = fc
    ynew = relabel[y64]
    yw = ynew // CPW
    ylid = (ynew % CPW).astype(np.float32)

    iota_arr = np.ascontiguousarray(
        np.tile(np.arange(CPW, dtype=np.float32), (128, 1))
    )

    hid_pads = []
    lids = []
    for k in range(NCORES):
        # pad slots duplicate row 0 (nonzero norm) with lid=-1 -> no effect
        hp = np.tile(hidden[0], (BS_PAD, 1))
        li = np.full((128, NT), -1.0, dtype=np.float32)
        hid_pads.append(hp)
        lids.append(li)

    for w in range(W):
        idxs = np.nonzero(yw == w)[0]
        for k in range(NCORES):
            sub = idxs[k::NCORES]
            n = len(sub)
            assert n <= 128 * TPW, f"window {w} core {k} overflow: {n}"
            i = np.arange(n)
            p = i // TPW
            j = w * TPW + (i % TPW)
            hid_pads[k][NT * p + j] = hidden[sub]
            lids[k][p, j] = ylid[sub]

    in_maps = []
    for k in range(NCORES):
        in_maps.append(
            {
                "hidden": np.ascontiguousarray(hid_pads[k]),
                "fc": fc_prime,
                "lid": np.ascontiguousarray(lids[k]),
                "iota": iota_arr,
            }
        )
    return in_maps


def finish(results) -> np.ndarray:
    tot = 0.0
    for r in results:
        tot += np.asarray(r["res"], dtype=np.float64).sum()
    return np.float32(1.0 + tot / (B * (C - 1)))


def kernel(hidden, feature_center, y) -> np.ndarray:
    from concourse.bass_utils import run_bass_kernel_spmd

    in_maps = make_in_maps(hidden, feature_center, y)
    nc = _get_nc()
    res = run_bass_kernel_spmd(nc, in_maps, core_ids=list(range(NCORES)))
    return finish(res.results)


# revision 9
# speedup vs baseline: 1.6638x; 1.2017x over previous
"""ContrastiveCenterLoss Trainium2 Bass kernel — gather-free formulation.

Math
----
reference:  dis[b,c] = cos(hidden_b, center_c);  intra_b = dis[b, y_b];
            inter_b  = (sum_c dis[b,c] - intra_b) / (C-1)
            loss     = mean(1 - intra_b + inter_b)

Folded form (exact algebraic identities):
    cn_c   = fc_c / ||fc_c||          (normalized centers)
    s      = sum_c cn_c
    X_b    = hidden_b / ||hidden_b||
    G_c    = sum_{b: y_b = c} X_b     (class-conditional sums)
    loss   = 1 + sum_c G_c . (s - C*cn_c) / (B * (C-1))

G is computed on the PE as one-hot matmuls: the HOST relabels classes
into 8 windows of 125 (balancing sample counts per window) and permutes
each core's batch shard so tile j (128 samples) only holds classes of
window j//5. Then per tile:  G_window += onehot[b,125]^T @ X[b,128].
The bf16 one-hot is precomputed on the host (pure index preprocessing)
and DMAed on an otherwise idle queue. Pad slots duplicate a real row
with an all-zero one-hot row, contributing exactly nothing.

Sharding: data-parallel over batch across 8 cores, fc replicated; host
sums the per-core partial reductions.
"""

import sys

sys.path.insert(0, "/opt/trn_rl_repo")

from contextlib import ExitStack

import numpy as np

import concourse.bass as bass
import concourse.tile as tile
from concourse import bacc, mybir

B, C, D = 32768, 1000, 128
NCORES = 8
W = 8                     # class windows
CPW = C // W              # 125 classes per window
TPW = 5                   # tiles per window
NT = W * TPW              # 40 tiles of 128 slots per core
BS_PAD = NT * 128         # 5120 padded slots per core
NG = 4                    # compute pipeline groups
TPG = NT // NG            # 10 tiles per group
F32 = mybir.dt.float32
BF16 = mybir.dt.bfloat16

_CACHED_NC = None


def build_nc() -> bass.Bass:
    AF = mybir.ActivationFunctionType
    OP = mybir.AluOpType

    nc = bacc.Bacc()
    hidden = nc.dram_tensor("hidden", [BS_PAD, D], F32, kind="ExternalInput")
    fc = nc.dram_tensor("fc", [C, D], F32, kind="ExternalInput")
    oh_t = nc.dram_tensor("oh", [128, NT, CPW], BF16, kind="ExternalInput")
    out_res = nc.dram_tensor("res", [128, W], F32, kind="ExternalOutput")

    with tile.TileContext(nc) as tc, ExitStack() as ctx:
        singles = ctx.enter_context(tc.tile_pool(name="singles", bufs=1))
        psum = ctx.enter_context(tc.tile_pool(name="psum", bufs=1, space="PSUM"))

        # ---------------- DMA in ----------------
        # fc on the scalar queue, hidden chunks on sync, one-hot on vector.
        fc_sb = singles.tile([CPW, W, D], F32)
        fc_src = fc[:, :].rearrange("(t p) d -> p t d", t=W)
        nc.scalar.dma_start(out=fc_sb[:, 0 : W // 2, :], in_=fc_src[:, 0 : W // 2, :])
        nc.scalar.dma_start(out=fc_sb[:, W // 2 :, :], in_=fc_src[:, W // 2 :, :])

        oh = singles.tile([128, NT, CPW], BF16)
        for g in range(NG):
            j0, j1 = g * TPG, (g + 1) * TPG
            nc.gpsimd.dma_start(out=oh[:, j0:j1, :], in_=oh_t[:, j0:j1, :])

        h_all = singles.tile([128, NT, D], F32)
        h_src = hidden[:, :].rearrange("(p i) d -> p i d", p=128)
        HCH = 8
        HTPC = NT // HCH
        for k in range(HCH):
            j0, j1 = k * HTPC, (k + 1) * HTPC
            nc.sync.dma_start(out=h_all[:, j0:j1, :], in_=h_src[:, j0:j1, :])

        # Preload the ACT sqrt/square table early (1.3us table load).
        warm = singles.tile([128, 1], F32)
        nc.scalar.activation(out=warm, in_=warm, func=AF.Sqrt, scale=0.0, bias=1.0)

        # ---------------- normalized centers ----------------
        sq_c = singles.tile([CPW, W, D], BF16)
        nc.scalar.activation(out=sq_c, in_=fc_sb, func=AF.Square)
        q_c = singles.tile([CPW, W], F32)
        nc.vector.tensor_reduce(
            out=q_c, in_=sq_c, axis=mybir.AxisListType.X, op=OP.add
        )
        rt_c = singles.tile([CPW, W], F32)
        nc.scalar.activation(out=rt_c, in_=q_c, func=AF.Sqrt)
        inv_c = singles.tile([CPW, W], F32)
        nc.vector.reciprocal(out=inv_c, in_=rt_c)

        cn_bf = singles.tile([CPW, W, D], BF16)
        nc.gpsimd.tensor_tensor(
            out=cn_bf,
            in0=fc_sb,
            in1=inv_c.broadcast_to([CPW, W, D]),
            op=OP.mult,
        )

        # s_rep[p, d] = sum_c cn[c, d] replicated on all 128 partitions:
        # 8 accumulating ones-matmuls (off critical path, PE idle early).
        ones_bf = singles.tile([CPW, 128], BF16)
        nc.vector.memset(ones_bf, 1.0)
        s_ps = psum.tile([128, D], F32)
        for t in range(W):
            nc.tensor.matmul(
                out=s_ps,
                lhsT=ones_bf,
                rhs=cn_bf[:, t, :],
                start=(t == 0),
                stop=(t == W - 1),
                skip_group_check=True,
            )

        # T = s - C*cn  (f32), the per-class loss weights
        T_sb = singles.tile([CPW, W, D], F32)
        nc.vector.scalar_tensor_tensor(
            out=T_sb,
            in0=cn_bf,
            scalar=float(-C),
            op0=OP.mult,
            in1=s_ps[0:CPW, :].unsqueeze(1).broadcast_to([CPW, W, D]),
            op1=OP.add,
        )

        # ---------------- main pipeline over 4 groups ----------------
        sq = singles.tile([128, NT, D], BF16)
        q_all = singles.tile([128, NT], F32)
        rt_all = singles.tile([128, NT], F32)
        invh = singles.tile([128, NT], F32)
        x_bf = singles.tile([128, NT, D], BF16)
        g_ps = psum.tile([CPW, W, D], F32)
        res_sb = singles.tile([128, W], F32)
        nc.vector.memset(res_sb, 0.0)
        scratch = singles.tile([CPW, W, D], F32)

        for g in range(NG):
            j0, j1 = g * TPG, (g + 1) * TPG
            nc.scalar.activation(
                out=sq[:, j0:j1, :], in_=h_all[:, j0:j1, :], func=AF.Square
            )
            nc.vector.tensor_reduce(
                out=q_all[:, j0:j1],
                in_=sq[:, j0:j1, :],
                axis=mybir.AxisListType.X,
                op=OP.add,
            )
            nc.scalar.activation(
                out=rt_all[:, j0:j1], in_=q_all[:, j0:j1], func=AF.Sqrt
            )
            nc.vector.reciprocal(out=invh[:, j0:j1], in_=rt_all[:, j0:j1])
            # X = h / ||h||  in bf16 (PE operand); alternate DVE/gpsimd
            eng = nc.vector if g % 2 == 0 else nc.gpsimd
            eng.tensor_tensor(
                out=x_bf[:, j0:j1, :],
                in0=h_all[:, j0:j1, :],
                in1=invh[:, j0:j1].broadcast_to([128, TPG, D]),
                op=OP.mult,
            )
            # G_window += onehot^T @ X   per tile
            for j in range(j0, j1):
                w = j // TPW
                nc.tensor.matmul(
                    out=g_ps[:, w, :],
                    lhsT=oh[:, j, :],
                    rhs=x_bf[:, j, :],
                    start=(j % TPW == 0),
                    stop=(j % TPW == TPW - 1),
                    skip_group_check=True,
                )
                if j % TPW == TPW - 1:
                    # window done: partial_w = sum_c G_w[c] . T_w[c]
                    nc.vector.scalar_tensor_tensor(
                        out=scratch[:, w, :],
                        in0=g_ps[:, w, :],
                        scalar=1.0,
                        op0=OP.mult,
                        in1=T_sb[:, w, :],
                        op1=OP.mult,
                        accum_out=res_sb[0:CPW, w : w + 1],
                    )

        nc.sync.dma_start(out=out_res[:, :], in_=res_sb)

    return nc


def _get_nc() -> bass.Bass:
    global _CACHED_NC
    if _CACHED_NC is None:
        _CACHED_NC = build_nc()
        _CACHED_NC.finalize()
    return _CACHED_NC


def make_in_maps(hidden, feature_center, y):
    import ml_dtypes

    hidden = np.ascontiguousarray(np.asarray(hidden), dtype=np.float32)
    fc = np.ascontiguousarray(np.asarray(feature_center), dtype=np.float32)
    y64 = np.asarray(y).astype(np.int64)

    counts = np.bincount(y64, minlength=C)
    order = np.argsort(-counts, kind="stable")
    # Greedy: heaviest class first into the lightest window with room.
    wsum = np.zeros(W, dtype=np.int64)
    wlen = np.zeros(W, dtype=np.int64)
    relabel = np.empty(C, dtype=np.int64)
    for c in order:
        cands = np.nonzero(wlen < CPW)[0]
        w = cands[np.argmin(wsum[cands])]
        relabel[c] = w * CPW + wlen[w]
        wlen[w] += 1
        wsum[w] += counts[c]

    fc_prime = np.empty_like(fc)
    fc_prime[relabel] = fc
    ynew = relabel[y64]
    yw = ynew // CPW
    ylid = ynew % CPW

    in_maps = []
    hid_pads = []
    ohs = []
    for k in range(NCORES):
        # pad slots duplicate row 0 (nonzero norm) with zero one-hot row
        hid_pads.append(np.tile(hidden[0], (BS_PAD, 1)))
        ohs.append(np.zeros((128, NT, CPW), dtype=np.float32))

    for w in range(W):
        idxs = np.nonzero(yw == w)[0]
        for k in range(NCORES):
            sub = idxs[k::NCORES]
            n = len(sub)
            assert n <= 128 * TPW, f"window {w} core {k} overflow: {n}"
            i = np.arange(n)
            p = i // TPW
            j = w * TPW + (i % TPW)
            hid_pads[k][NT * p + j] = hidden[sub]
            ohs[k][p, j, ylid[sub]] = 1.0

    for k in range(NCORES):
        in_maps.append(
            {
                "hidden": np.ascontiguousarray(hid_pads[k]),
                "fc": fc_prime,
                "oh": ohs[k].astype(ml_dtypes.bfloat16),
            }
        )
    return in_maps


def finish(results) -> np.ndarray:
    tot = 0.0
    for r in results:
        tot += np.asarray(r["res"], dtype=np.float64).sum()
    return np.float32(1.0 + tot / (B * (C - 1)))


def kernel(hidden, feature_center, y) -> np.ndarray:
    from concourse.bass_utils import run_bass_kernel_spmd

    in_maps = make_in_maps(hidden, feature_center, y)
    nc = _get_nc()
    res = run_bass_kernel_spmd(nc, in_maps, core_ids=list(range(NCORES)))
    return finish(res.results)
